# revision 28
# baseline (speedup 1.0000x reference)
"""Trainium2 Bass kernel for nn_Attention_4063039062503.

Reference (per batch b, C=128 channels, N=4096 points):
    q = W1 @ x + b1;  k = W2 @ x + b2          # [C, N]
    s[n, m] = q[:, n] . k[:, m]                # [N, N]
    a = softmax(s, axis=m)
    out = relu(x + x @ a.T)                    # out[:, n] = x @ a[n, :]

Sharding: 8 cores, core i -> batch i//2, query half i%2 (2048 queries),
full 4096 keys local (no collectives).

Per-core plan (flash-attention style, scores never leave the chip):
  - Q/K projections + S^T score tiles [m=128, q] on TensorE in fp16
    (10-bit mantissa; simulated end-to-end error 2.7e-3 vs 2e-2 gate)
  - exp(s - 30) on ScalarE PSUM->SBUF, bf16 out (constant shift is exact
    for softmax and keeps e^s in fp32/bf16 range for scores up to ~+118)
  - O[c, q] += xT[m-tile] @ E on TensorE in bf16, fp32 PSUM accumulation
  - row-sums: DVE bf16 accumulate across m-tiles + one ones-matmul
  - normalize via reciprocal_approx_fast + fp32 broadcast-matmul, then
    residual add (fp32 x) + relu on DVE, DMA out
  - O-matmuls software-pipelined DLAG iterations behind the S/exp stream
    so their ACT-waits are pre-satisfied and the PE queue stays dense
"""
from contextlib import ExitStack

import numpy as np
import ml_dtypes

import concourse.bass as bass
import concourse.tile as tile
from concourse import bacc, mybir
from concourse.bass_utils import run_bass_kernel_spmd

B = 4
C = 128
N = 4096            # keys per batch
NQ = 2048           # queries per core
QB = 512            # query block (PSUM bank free size)
MT = 128            # m (key) tile
N_MT = N // MT      # 32
N_QB = NQ // QB     # 4
DLAG = 3            # O-matmul lag (in m-tiles) behind the S/exp pipeline

# packed fp16 input layout: [128, XW_COLS]
XK_OFS = 0                  # x full        [128, 4096]
XQ_OFS = N                  # x query half  [128, 2048]
W1T_OFS = XQ_OFS + NQ       # W1.T          [128, 128]
W2T_OFS = W1T_OFS + C       # W2.T          [128, 128]
B1_OFS = W2T_OFS + C        # b1 column     [128, 1]
B2_OFS = B1_OFS + 1         # b2 column     [128, 1]
XW_COLS = B2_OFS + 1

F32 = mybir.dt.float32
F16 = mybir.dt.float16
BF16 = mybir.dt.bfloat16


def build_nc():
    nc = bacc.Bacc("TRN2", target_bir_lowering=False, debug=False, num_devices=8)
    xw_ext = nc.declare_dram_parameter("xw", [C, XW_COLS], F16, isOutput=False)
    xt_ext = nc.declare_dram_parameter("xt", [C, N], BF16, isOutput=False)
    xr_ext = nc.declare_dram_parameter("xr", [C, NQ + 2], F32, isOutput=False)
    out_ext = nc.declare_dram_parameter("out", [C, NQ], F32, isOutput=True)

    with ExitStack() as ctx:
        tc = ctx.enter_context(tile.TileContext(nc))
        consts = ctx.enter_context(tc.tile_pool(name="consts", bufs=1))
        sb_in = ctx.enter_context(tc.tile_pool(name="sb_in", bufs=1))
        sb_kq = ctx.enter_context(tc.tile_pool(name="sb_kq", bufs=1))
        sb_e = ctx.enter_context(tc.tile_pool(name="sb_e", bufs=1))
        sb_acc = ctx.enter_context(tc.tile_pool(name="sb_acc", bufs=2))
        sb_tail = ctx.enter_context(tc.tile_pool(name="sb_tail", bufs=2))
        ps_s = ctx.enter_context(tc.tile_pool(name="ps_s", bufs=2, space="PSUM"))
        ps_o = ctx.enter_context(tc.tile_pool(name="ps_o", bufs=2, space="PSUM"))
        ps_r = ctx.enter_context(tc.tile_pool(name="ps_r", bufs=2, space="PSUM"))


        ones_bf = consts.tile([C, C], BF16, tag="ones_bf")
        nc.vector.memset(ones_bf[:], 1.0)
        shift = consts.tile([C, 1], F32, tag="shift")
        nc.vector.memset(shift[:], -30.0)
        # warm the exp table early (ACT_TABLE_LOAD ~2.7us)
        warm = consts.tile([1, 16], F32, tag="warm")
        nc.vector.memset(warm[:], 0.0)
        warm_o = consts.tile([1, 16], F32, tag="warm_o")
        nc.scalar.activation(warm_o[:], warm[:], mybir.ActivationFunctionType.Exp)

        xw = sb_in.tile([C, XW_COLS], F16, tag="xw")
        xt = sb_in.tile([C, N], BF16, tag="xt")
        xr = sb_in.tile([C, NQ + 2], F32, tag="xr")
        # chunked input DMAs, ordered so the first S-matmuls start early:
        # weights, first q chunks, k chunks, rest
        nc.sync.dma_start(xw[:, W1T_OFS:XW_COLS], xw_ext[:, W1T_OFS:XW_COLS])
        nc.sync.dma_start(xr[:, NQ:NQ + 2], xr_ext[:, NQ:NQ + 2])
        for j in range(NQ // QB):
            nc.sync.dma_start(xw[:, XQ_OFS + j * QB:XQ_OFS + (j + 1) * QB],
                              xw_ext[:, XQ_OFS + j * QB:XQ_OFS + (j + 1) * QB])
        for j in range(N // QB):
            nc.sync.dma_start(xw[:, j * QB:(j + 1) * QB],
                              xw_ext[:, j * QB:(j + 1) * QB])
        for j in range(4):
            nc.sync.dma_start(xt[:, j * (N // 4):(j + 1) * (N // 4)],
                              xt_ext[:, j * (N // 4):(j + 1) * (N // 4)])
        nc.sync.dma_start(xr[:, 0:NQ], xr_ext[:, 0:NQ])

        kt = sb_kq.tile([C, N], F16, tag="kt")       # K = W2 x + b2
        qt = sb_kq.tile([C, NQ], F16, tag="qt")      # Q = W1 x + b1 (query half)

        _evac_flip = [False]

        def proj(dst, w_ofs, b_col, x_ofs, j):
            ps = ps_s.tile([C, QB], F32, tag="s")
            nc.tensor.matmul(ps[:], xw[:, w_ofs:w_ofs + C],
                             xw[:, x_ofs + j * QB:x_ofs + (j + 1) * QB],
                             start=True, stop=True)
            # alternate PSUM evacuation between DVE and ScalarE so the
            # projection phase isn't serialized on one engine
            _evac_flip[0] = not _evac_flip[0]
            if _evac_flip[0]:
                nc.vector.tensor_scalar(
                    out=dst[:, j * QB:(j + 1) * QB], in0=ps[:],
                    scalar1=xr[:, NQ + b_col:NQ + b_col + 1], scalar2=None,
                    op0=mybir.AluOpType.add)
            else:
                nc.scalar.activation(
                    dst[:, j * QB:(j + 1) * QB], ps[:],
                    mybir.ActivationFunctionType.Identity,
                    bias=xr[:, NQ + b_col:NQ + b_col + 1])

        # interleave so kt/qt chunks needed first are produced first
        proj(qt, W1T_OFS, 0, XQ_OFS, 0)
        proj(qt, W1T_OFS, 0, XQ_OFS, 1)
        proj(kt, W2T_OFS, 1, XK_OFS, 0)
        proj(kt, W2T_OFS, 1, XK_OFS, 1)
        proj(qt, W1T_OFS, 0, XQ_OFS, 2)
        proj(qt, W1T_OFS, 0, XQ_OFS, 3)
        for j in range(2, N // QB):
            proj(kt, W2T_OFS, 1, XK_OFS, j)

        # E staged for a whole pass in SBUF so O-matmuls can lag
        e_stage = sb_e.tile([C, N_MT * 2 * QB], BF16, tag="e")

        # two passes, each covering a pair of query blocks (2*QB = 1024 q)
        for p in range(N_QB // 2):
            q0 = 2 * p * QB                      # col offset of this q-pair
            o_psA = ps_o.tile([C, QB], F32, tag="o")
            o_psB = ps_o.tile([C, QB], F32, tag="o")
            acc = sb_acc.tile([C, 2 * QB], BF16, tag="acc")

            def do_s(mt):
                s_ps = ps_s.tile([C, 2 * QB], F32, tag="s")
                for j in range(2):
                    nc.tensor.matmul(
                        s_ps[:, j * QB:(j + 1) * QB],
                        kt[:, mt * MT:(mt + 1) * MT],
                        qt[:, q0 + j * QB:q0 + (j + 1) * QB],
                        start=True, stop=True)
                e_g = e_stage[:, mt * 2 * QB:(mt + 1) * 2 * QB]
                nc.scalar.activation(e_g, s_ps[:],
                                     mybir.ActivationFunctionType.Exp,
                                     bias=shift[:, 0:1])
                if mt == 0:
                    nc.vector.tensor_copy(acc[:], e_g)
                else:
                    nc.vector.tensor_tensor(acc[:], acc[:], e_g,
                                            op=mybir.AluOpType.add)

            def do_o(mt):
                for j, o_ps in enumerate((o_psA, o_psB)):
                    nc.tensor.matmul(
                        o_ps[:],
                        xt[:, mt * MT:(mt + 1) * MT],
                        e_stage[:, (mt * 2 + j) * QB:(mt * 2 + j + 1) * QB],
                        start=(mt == 0), stop=(mt == N_MT - 1))

            for mt in range(N_MT + DLAG):
                if mt < N_MT:
                    do_s(mt)
                if mt >= DLAG:
                    do_o(mt - DLAG)

            # per-qb tail: row-sum -> reciprocal -> broadcast -> norm+residual+relu
            for j, o_ps in enumerate((o_psA, o_psB)):
                qofs = q0 + j * QB
                # ones[128,128] stationary: every output partition gets the
                # row-sum -> broadcast comes free with the reduction matmul
                r_ps = ps_r.tile([C, QB], F32, tag="r")
                nc.tensor.matmul(r_ps[:], ones_bf[:], acc[:, j * QB:(j + 1) * QB],
                                 start=True, stop=True)
                bc = sb_tail.tile([C, QB], F32, tag="bcs")
                nc.vector.reciprocal_approx_fast(bc[:], r_ps[:])
                t2 = sb_tail.tile([C, QB], F32, tag="t2")
                nc.vector.tensor_tensor(t2[:], o_ps[:], bc[:],
                                        op=mybir.AluOpType.mult)
                t3 = sb_tail.tile([C, QB], F32, tag="t3")
                nc.vector.tensor_tensor(t3[:], t2[:], xr[:, qofs:qofs + QB],
                                        op=mybir.AluOpType.add)
                o_out = sb_tail.tile([C, QB], F32, tag="o_out")
                nc.vector.tensor_scalar_max(o_out[:], t3[:], 0.0)
                nc.sync.dma_start(out_ext[:, qofs:qofs + QB], o_out[:])

    nc.compile()
    return nc


_NC_CACHE = None


def _get_nc():
    global _NC_CACHE
    if _NC_CACHE is None:
        _NC_CACHE = build_nc()
    return _NC_CACHE


def make_in_maps(x, W1, b1, W2, b2):
    x = np.asarray(x, np.float32)
    W1 = np.asarray(W1, np.float32)
    b1 = np.asarray(b1, np.float32)
    W2 = np.asarray(W2, np.float32)
    b2 = np.asarray(b2, np.float32)
    in_maps = []
    for core in range(8):
        b, h = divmod(core, 2)
        xb = x[b]                                    # [128, 4096]
        xq = xb[:, h * NQ:(h + 1) * NQ]
        xw = np.empty((C, XW_COLS), np.float16)
        xw[:, XK_OFS:XK_OFS + N] = xb
        xw[:, XQ_OFS:XQ_OFS + NQ] = xq
        xw[:, W1T_OFS:W1T_OFS + C] = W1.T
        xw[:, W2T_OFS:W2T_OFS + C] = W2.T
        xw[:, B1_OFS] = b1
        xw[:, B2_OFS] = b2
        # xt[:, mt*128 + c] = x[b].T[mt*128 + (partition), c]
        xt = np.ascontiguousarray(
            xb.T.reshape(N_MT, MT, C).transpose(1, 0, 2).reshape(MT, N_MT * C)
        ).astype(ml_dtypes.bfloat16)
        xrr = np.empty((C, NQ + 2), np.float32)
        xrr[:, :NQ] = xq
        xrr[:, NQ] = b1
        xrr[:, NQ + 1] = b2
        in_maps.append({"xw": xw, "xt": xt, "xr": xrr})
    return in_maps


def run(x, W1, b1, W2, b2, trace=False):
    nc = _get_nc()
    in_maps = make_in_maps(x, W1, b1, W2, b2)
    last_err = None
    for _attempt in range(3):
        try:
            res = run_bass_kernel_spmd(nc, in_maps, core_ids=list(range(8)),
                                       trace=trace)
            break
        except Exception as e:  # transient NRT/device errors: retry
            last_err = e
    else:
        raise last_err
    out = np.empty((B, C, N), np.float32)
    for core in range(8):
        b, h = divmod(core, 2)
        out[b][:, h * NQ:(h + 1) * NQ] = res.results[core]["out"]
    return out, res


def kernel(x, W1, b1, W2, b2):
    out, _ = run(x, W1, b1, W2, b2, trace=False)
    return out


# revision 29
# speedup vs baseline: 1.0121x; 1.0121x over previous
"""Trainium2 Bass kernel for nn_Attention_4063039062503.

Reference (per batch b, C=128 channels, N=4096 points):
    q = W1 @ x + b1;  k = W2 @ x + b2          # [C, N]
    s[n, m] = q[:, n] . k[:, m]                # [N, N]
    a = softmax(s, axis=m)
    out = relu(x + x @ a.T)                    # out[:, n] = x @ a[n, :]

Sharding: 8 cores, core i -> batch i//2, query half i%2 (2048 queries),
full 4096 keys local (no collectives).

Per-core plan (flash-attention style, scores never leave the chip):
  - Q/K projections + S^T score tiles [m=128, q] on TensorE in fp16
    (10-bit mantissa; simulated end-to-end error 2.7e-3 vs 2e-2 gate)
  - exp(s - 30) on ScalarE PSUM->SBUF, bf16 out (constant shift is exact
    for softmax and keeps e^s in fp32/bf16 range for scores up to ~+118)
  - O[c, q] += xT[m-tile] @ E on TensorE in bf16, fp32 PSUM accumulation
  - row-sums: DVE bf16 accumulate across m-tiles + one ones-matmul
  - normalize via reciprocal_approx_fast + fp32 broadcast-matmul, then
    residual add (fp32 x) + relu on DVE, DMA out
  - O-matmuls software-pipelined DLAG iterations behind the S/exp stream
    so their ACT-waits are pre-satisfied and the PE queue stays dense
"""
from contextlib import ExitStack

import numpy as np
import ml_dtypes

import concourse.bass as bass
import concourse.tile as tile
from concourse import bacc, mybir
from concourse.bass_utils import run_bass_kernel_spmd

B = 4
C = 128
N = 4096            # keys per batch
NQ = 2048           # queries per core
QB = 512            # query block (PSUM bank free size)
MT = 128            # m (key) tile
N_MT = N // MT      # 32
N_QB = NQ // QB     # 4
DLAG = 3            # O-matmul lag (in m-tiles) behind the S/exp pipeline

# packed fp16 input layout: [128, XW_COLS]
XK_OFS = 0                  # x full        [128, 4096]
XQ_OFS = N                  # x query half  [128, 2048]
W1T_OFS = XQ_OFS + NQ       # W1.T          [128, 128]
W2T_OFS = W1T_OFS + C       # W2.T          [128, 128]
B1_OFS = W2T_OFS + C        # b1 column     [128, 1]
B2_OFS = B1_OFS + 1         # b2 column     [128, 1]
XW_COLS = B2_OFS + 1

F32 = mybir.dt.float32
F16 = mybir.dt.float16
BF16 = mybir.dt.bfloat16


def build_nc():
    nc = bacc.Bacc("TRN2", target_bir_lowering=False, debug=False, num_devices=8)
    xw_ext = nc.declare_dram_parameter("xw", [C, XW_COLS], F16, isOutput=False)
    xt_ext = nc.declare_dram_parameter("xt", [C, N], BF16, isOutput=False)
    xr_ext = nc.declare_dram_parameter("xr", [C, NQ + 2], F32, isOutput=False)
    out_ext = nc.declare_dram_parameter("out", [C, NQ], F32, isOutput=True)

    with ExitStack() as ctx:
        tc = ctx.enter_context(tile.TileContext(nc))
        consts = ctx.enter_context(tc.tile_pool(name="consts", bufs=1))
        sb_in = ctx.enter_context(tc.tile_pool(name="sb_in", bufs=1))
        sb_kq = ctx.enter_context(tc.tile_pool(name="sb_kq", bufs=1))
        sb_e = ctx.enter_context(tc.tile_pool(name="sb_e", bufs=1))
        sb_acc = ctx.enter_context(tc.tile_pool(name="sb_acc", bufs=2))
        sb_tail = ctx.enter_context(tc.tile_pool(name="sb_tail", bufs=2))
        ps_s = ctx.enter_context(tc.tile_pool(name="ps_s", bufs=2, space="PSUM"))
        ps_o = ctx.enter_context(tc.tile_pool(name="ps_o", bufs=2, space="PSUM"))
        ps_r = ctx.enter_context(tc.tile_pool(name="ps_r", bufs=2, space="PSUM"))


        ones_bf = consts.tile([C, C], BF16, tag="ones_bf")
        nc.vector.memset(ones_bf[:], 1.0)
        shift = consts.tile([C, 1], F32, tag="shift")
        nc.vector.memset(shift[:], -30.0)
        # warm the exp table early (ACT_TABLE_LOAD ~2.7us)
        warm = consts.tile([1, 16], F32, tag="warm")
        nc.vector.memset(warm[:], 0.0)
        warm_o = consts.tile([1, 16], F32, tag="warm_o")
        nc.scalar.activation(warm_o[:], warm[:], mybir.ActivationFunctionType.Exp)

        xw = sb_in.tile([C, XW_COLS], F16, tag="xw")
        xt = sb_in.tile([C, N], BF16, tag="xt")
        xr = sb_in.tile([C, NQ + 2], F32, tag="xr")
        # chunked input DMAs, ordered so the first S-matmuls start early:
        # weights, first q chunks, k chunks, rest
        nc.sync.dma_start(xw[:, W1T_OFS:XW_COLS], xw_ext[:, W1T_OFS:XW_COLS])
        nc.sync.dma_start(xr[:, NQ:NQ + 2], xr_ext[:, NQ:NQ + 2])
        for j in range(NQ // (2 * QB)):
            nc.sync.dma_start(
                xw[:, XQ_OFS + j * 2 * QB:XQ_OFS + (j + 1) * 2 * QB],
                xw_ext[:, XQ_OFS + j * 2 * QB:XQ_OFS + (j + 1) * 2 * QB])
        for j in range(N // (2 * QB)):
            nc.sync.dma_start(xw[:, j * 2 * QB:(j + 1) * 2 * QB],
                              xw_ext[:, j * 2 * QB:(j + 1) * 2 * QB])
        nc.sync.dma_start(xt[:], xt_ext[:])
        nc.sync.dma_start(xr[:, 0:NQ], xr_ext[:, 0:NQ])

        kt = sb_kq.tile([C, N], F16, tag="kt")       # K = W2 x + b2
        qt = sb_kq.tile([C, NQ], F16, tag="qt")      # Q = W1 x + b1 (query half)

        _evac_flip = [False]

        def proj(dst, w_ofs, b_col, x_ofs, j):
            ps = ps_s.tile([C, QB], F32, tag="s")
            nc.tensor.matmul(ps[:], xw[:, w_ofs:w_ofs + C],
                             xw[:, x_ofs + j * QB:x_ofs + (j + 1) * QB],
                             start=True, stop=True)
            # alternate PSUM evacuation between DVE and ScalarE so the
            # projection phase isn't serialized on one engine
            _evac_flip[0] = not _evac_flip[0]
            if _evac_flip[0]:
                nc.vector.tensor_scalar(
                    out=dst[:, j * QB:(j + 1) * QB], in0=ps[:],
                    scalar1=xr[:, NQ + b_col:NQ + b_col + 1], scalar2=None,
                    op0=mybir.AluOpType.add)
            else:
                nc.scalar.activation(
                    dst[:, j * QB:(j + 1) * QB], ps[:],
                    mybir.ActivationFunctionType.Identity,
                    bias=xr[:, NQ + b_col:NQ + b_col + 1])

        # interleave so kt/qt chunks needed first are produced first
        proj(qt, W1T_OFS, 0, XQ_OFS, 0)
        proj(qt, W1T_OFS, 0, XQ_OFS, 1)
        proj(kt, W2T_OFS, 1, XK_OFS, 0)
        proj(kt, W2T_OFS, 1, XK_OFS, 1)
        proj(qt, W1T_OFS, 0, XQ_OFS, 2)
        proj(qt, W1T_OFS, 0, XQ_OFS, 3)
        for j in range(2, N // QB):
            proj(kt, W2T_OFS, 1, XK_OFS, j)

        # E staged for a whole pass in SBUF so O-matmuls can lag
        e_stage = sb_e.tile([C, N_MT * 2 * QB], BF16, tag="e")

        # two passes, each covering a pair of query blocks (2*QB = 1024 q)
        for p in range(N_QB // 2):
            q0 = 2 * p * QB                      # col offset of this q-pair
            o_psA = ps_o.tile([C, QB], F32, tag="o")
            o_psB = ps_o.tile([C, QB], F32, tag="o")
            acc = sb_acc.tile([C, 2 * QB], BF16, tag="acc")

            def do_s(mt):
                s_ps = ps_s.tile([C, 2 * QB], F32, tag="s")
                for j in range(2):
                    nc.tensor.matmul(
                        s_ps[:, j * QB:(j + 1) * QB],
                        kt[:, mt * MT:(mt + 1) * MT],
                        qt[:, q0 + j * QB:q0 + (j + 1) * QB],
                        start=True, stop=True)
                e_g = e_stage[:, mt * 2 * QB:(mt + 1) * 2 * QB]
                nc.scalar.activation(e_g, s_ps[:],
                                     mybir.ActivationFunctionType.Exp,
                                     bias=shift[:, 0:1])
                if mt == 0:
                    nc.vector.tensor_copy(acc[:], e_g)
                else:
                    nc.vector.tensor_tensor(acc[:], acc[:], e_g,
                                            op=mybir.AluOpType.add)

            def do_o(mt):
                for j, o_ps in enumerate((o_psA, o_psB)):
                    nc.tensor.matmul(
                        o_ps[:],
                        xt[:, mt * MT:(mt + 1) * MT],
                        e_stage[:, (mt * 2 + j) * QB:(mt * 2 + j + 1) * QB],
                        start=(mt == 0), stop=(mt == N_MT - 1))

            for mt in range(N_MT + DLAG):
                if mt < N_MT:
                    do_s(mt)
                if mt >= DLAG:
                    do_o(mt - DLAG)

            # per-qb tail: row-sum -> reciprocal -> broadcast -> norm+residual+relu
            for j, o_ps in enumerate((o_psA, o_psB)):
                qofs = q0 + j * QB
                # ones[128,128] stationary: every output partition gets the
                # row-sum -> broadcast comes free with the reduction matmul
                r_ps = ps_r.tile([C, QB], F32, tag="r")
                nc.tensor.matmul(r_ps[:], ones_bf[:], acc[:, j * QB:(j + 1) * QB],
                                 start=True, stop=True)
                bc = sb_tail.tile([C, QB], F32, tag="bcs")
                nc.vector.reciprocal_approx_fast(bc[:], r_ps[:])
                t2 = sb_tail.tile([C, QB], F32, tag="t2")
                nc.vector.tensor_tensor(t2[:], o_ps[:], bc[:],
                                        op=mybir.AluOpType.mult)
                t3 = sb_tail.tile([C, QB], F32, tag="t3")
                nc.vector.tensor_tensor(t3[:], t2[:], xr[:, qofs:qofs + QB],
                                        op=mybir.AluOpType.add)
                o_out = sb_tail.tile([C, QB], F32, tag="o_out")
                nc.vector.tensor_scalar_max(o_out[:], t3[:], 0.0)
                nc.sync.dma_start(out_ext[:, qofs:qofs + QB], o_out[:])

    nc.compile()
    return nc


_NC_CACHE = None


def _get_nc():
    global _NC_CACHE
    if _NC_CACHE is None:
        _NC_CACHE = build_nc()
    return _NC_CACHE


def make_in_maps(x, W1, b1, W2, b2):
    x = np.asarray(x, np.float32)
    W1 = np.asarray(W1, np.float32)
    b1 = np.asarray(b1, np.float32)
    W2 = np.asarray(W2, np.float32)
    b2 = np.asarray(b2, np.float32)
    in_maps = []
    for core in range(8):
        b, h = divmod(core, 2)
        xb = x[b]                                    # [128, 4096]
        xq = xb[:, h * NQ:(h + 1) * NQ]
        xw = np.empty((C, XW_COLS), np.float16)
        xw[:, XK_OFS:XK_OFS + N] = xb
        xw[:, XQ_OFS:XQ_OFS + NQ] = xq
        xw[:, W1T_OFS:W1T_OFS + C] = W1.T
        xw[:, W2T_OFS:W2T_OFS + C] = W2.T
        xw[:, B1_OFS] = b1
        xw[:, B2_OFS] = b2
        # xt[:, mt*128 + c] = x[b].T[mt*128 + (partition), c]
        xt = np.ascontiguousarray(
            xb.T.reshape(N_MT, MT, C).transpose(1, 0, 2).reshape(MT, N_MT * C)
        ).astype(ml_dtypes.bfloat16)
        xrr = np.empty((C, NQ + 2), np.float32)
        xrr[:, :NQ] = xq
        xrr[:, NQ] = b1
        xrr[:, NQ + 1] = b2
        in_maps.append({"xw": xw, "xt": xt, "xr": xrr})
    return in_maps


def run(x, W1, b1, W2, b2, trace=False):
    nc = _get_nc()
    in_maps = make_in_maps(x, W1, b1, W2, b2)
    last_err = None
    for _attempt in range(3):
        try:
            res = run_bass_kernel_spmd(nc, in_maps, core_ids=list(range(8)),
                                       trace=trace)
            break
        except Exception as e:  # transient NRT/device errors: retry
            last_err = e
    else:
        raise last_err
    out = np.empty((B, C, N), np.float32)
    for core in range(8):
        b, h = divmod(core, 2)
        out[b][:, h * NQ:(h + 1) * NQ] = res.results[core]["out"]
    return out, res


def kernel(x, W1, b1, W2, b2):
    out, _ = run(x, W1, b1, W2, b2, trace=False)
    return out
